# revision 10
# baseline (speedup 1.0000x reference)
"""Single-head causal self-attention (B=4, T=4096, C=1024, HS=64) on 8 TRN2 cores.

Sharding: core = 2*b + h; the two cores of batch b split the 8 query blocks
(512 rows each) in a load-balanced interleave: h=0 -> blocks {0,3,4,7},
h=1 -> blocks {1,2,5,6}.  Slot j = query block g_j (base q-tile P0 = 4*g_j);
q-tile P (global 128-row tile) attends to context chunks k = 0..P (exact
causal, 128-key chunks).  h=1 never attends past chunk 27, so it skips
loading/projecting context block 7 entirely.

Layouts chosen so every matmul's *output free size* (the only thing the PE
charges for) is minimal:
  A (non-owned block): [K^T|V^T] = ([Wk|Wv]).T @ xt  (one PSUM bank, K^T
     copied to kv_sb rows 0:64; V^T rows PE-transposed to V natural in vp)
  B (owned block): [K^T|Q^T] = ([Wk|Wq/8]).T @ xt    (same bank; K^T ->
     kv_sb, Q^T -> qT) and V natural directly: V[keys,64] += xt_chunk.T @
     Wv_chunk (free size 64 per chunk-tile, half the cost of a second
     512-wide pack + transpose)
  S: S^T chunk [128k, w] = kT_chunk.T @ qT  (w = 512-128*max(0,k-P0))
  E: exp on ScalarE, PSUM->SBUF bf16; full-width chunk pairs fused into one
     activation; only diagonal chunks (k in [P0, P0+3]) need the shared
     128x128 triangular mask
  O: O[q,65] += E_piece.T @ [V|1]_chunk   (65-wide moving -> cheap PE)
  F: out = O[:, :64] * (1/O[:, 64]) per q-tile, written [128, 16, 64] f32

DMA: transfers serialize on the shared DMA-engine pool, but each DMACopy
also costs ~1.3us of DGE/seq overhead on its queue, so context blocks
alternate between the SP (HWDGE) and Pool (SWDGE) queues; weights+mask ride
two packed copies on the otherwise-idle Activation queue (the first carries
only what position 0 needs).  Context blocks stream in an order that keeps
all four slots supplied (owned blocks early); attention units (chunk pairs /
diagonal singles) emit as soon as their operands are resident, in any chunk
order (PSUM accumulation commutes; the first/last emitted matmul per q-tile
carries start/stop).  PE warms its clock on dummy matmuls over a memset
scrap tile (no dependency on identity generation) until the first real
operands land.
"""

import numpy as np
import ml_dtypes

B, T, C, HS = 4, 4096, 1024, 64
NSLOT = 4
CCH = C // 128
NCHUNK = T // 128        # 32 context chunks

CONFIG = {
    0: dict(blocks=[0, 3, 4, 7],
            sp=[[3], [2], [1], [6]], pool=[[4], [7], [5]],
            border=[0, 3, 4, 2, 7, 1, 5, 6], tail_slot=3,
            cap=[3, 4, 5, 5, 6, 6, 7, 8], warm=128),
    1: dict(blocks=[1, 2, 5, 6],           # block 7 context unused: skip it
            sp=[[2], [0], [6]], pool=[[5], [3], [4]],
            border=[1, 2, 5, 0, 3, 6, 4], tail_slot=3,
            cap=[3, 4, 5, 5, 6, 6, 7, 8], warm=128),
}

_programs = {}


def _build_program(blocks, border, sp_q, pool_q, tail_slot=None, cap=99,
                   warm_n=128):
    import concourse.mybir as mybir
    import concourse.tile as tile
    from concourse import bacc
    from concourse.masks import make_identity
    from contextlib import ExitStack

    f32 = mybir.dt.float32
    bf16 = mybir.dt.bfloat16

    P0 = [4 * g for g in blocks]          # base q-tile per slot
    npos = len(border)
    nc = bacc.Bacc("TRN2", target_bir_lowering=False, debug=False, num_devices=8)

    xt_d = nc.dram_tensor("xt", [C, T], bf16, kind="ExternalInput").ap()
    wp_d = nc.dram_tensor("wpack", [128, 21, 128], bf16, kind="ExternalInput").ap()
    out_d = nc.dram_tensor("out", [128, 16, HS], bf16, kind="ExternalOutput").ap()

    with tile.TileContext(nc) as tc, ExitStack() as ctx:
        consts = ctx.enter_context(tc.tile_pool(name="consts", bufs=1))
        epool = ctx.enter_context(tc.tile_pool(name="epool", bufs=1))
        mpool = ctx.enter_context(tc.tile_pool(name="mpool", bufs=2))

        xt = consts.tile([128, CCH, T], bf16)
        wp = consts.tile([128, 21, 128], bf16)
        kv_sb = consts.tile([128, 8, 512], bf16)   # rows 0:64 K^T; 64:128 V^T
        qT = consts.tile([64, NSLOT, 512], bf16)
        vp = consts.tile([128, NCHUNK, HS + 1], bf16)  # [V | ones]
        out_sb = consts.tile([128, 16, HS], bf16)
        id_hi = consts.tile([128, 64], bf16)   # identity on partitions 64:128
        warm_src = consts.tile([64, 64], bf16)

        # weight pack layout along dim 1:
        #   0..7   [Wk | Wq/8]  (owned-block stat)
        #   8      tri mask
        #   9..16  [Wk | Wv]    (non-owned-block stat)
        #   17..20 Wv moving chunks ([128, 8, 64] packed two per 128-col slot)
        def wqk_ap(ci):
            return wp[:, ci, :]

        def wkv_ap(ci):
            return wp[:, 9 + ci, :]

        def wv_mov(ci):
            return wp[:, 17 + ci // 2, 64 * (ci % 2):64 * (ci % 2) + 64]

        tri = wp[:, 8, :]

        # warmup scrap first so the PE clock-ramp matmuls depend on nothing
        # but one cheap DVE memset
        nc.vector.memset(warm_src, 0.0)
        # weights+mask on the (idle until exp) Act queue: first copy carries
        # only what position 0's [K^T|Q^T] pack needs; the non-owned pack
        # follows.  Wv-moving rides the Pool queue right behind the first
        # context half so V-natural of block 0 never waits on it.
        nc.scalar.dma_start(out=wp[:, 0:9, :], in_=wp_d[:, 0:9, :])
        nc.scalar.dma_start(out=wp[:, 9:17, :], in_=wp_d[:, 9:17, :])
        nc.vector.memset(vp[:, :, HS], 1.0)

        # context block loads: first block split in halves across SP/Pool,
        # remaining blocks as contiguous runs (one copy per run — each copy
        # costs ~1.3us of queue-sequencer/DGE overhead, so fewer is faster)
        xt_r = xt_d.rearrange("(a p) t -> p a t", p=128)

        def ld(eng, lo, hi):
            sl = slice(lo, hi)
            eng.dma_start(out=xt[:, :, sl], in_=xt_r[:, :, sl])

        g0 = border[0]
        ld(nc.sync, g0 * 512, g0 * 512 + 256)
        ld(nc.gpsimd, g0 * 512 + 256, g0 * 512 + 512)
        # Wv-moving rides SP right behind the first context half so the
        # V-natural of block 0 gets it in time (the pool queue's SWDGE gens
        # are slower and would let later context blocks jump it on the
        # shared DMA-engine pipe)
        nc.sync.dma_start(out=wp[:, 17:21, :], in_=wp_d[:, 17:21, :])
        for run in sp_q:
            ld(nc.sync, run[0] * 512, (run[-1] + 1) * 512)
        for run in pool_q:
            ld(nc.gpsimd, run[0] * 512, (run[-1] + 1) * 512)
        # identity (for the V^T transposes of non-owned blocks) runs on the
        # Pool engine: build it AFTER the pool-queue DMA issues or it delays
        # every SWDGE descriptor generation by ~3us
        make_identity(nc, id_hi[64:128, :])

        with tc.tile_pool(name="psA", bufs=1, space="PSUM") as psA, \
             tc.tile_pool(name="psC", bufs=2, space="PSUM") as psC, \
             tc.tile_pool(name="psO", bufs=1, space="PSUM") as psO:

            # 16 persistent O accumulators packed into 3 PSUM banks, grouped
            # by finalize time (PSUM WAR deps are tile-granular: a finalize
            # read blocks later accumulation into the same bank, so the
            # last-finishing slot 3 gets a pure bank)
            acc = [psO.tile([128, 7, HS + 1], f32, tag="acc0", name="acc0"),
                   psO.tile([128, 5, HS + 1], f32, tag="acc1", name="acc1"),
                   psO.tile([128, 4, HS + 1], f32, tag="acc2", name="acc2")]

            def acc_ap(tau):
                if tau < 7:
                    return acc[0][:, tau, :]
                if tau < 12:
                    return acc[1][:, tau - 7, :]
                return acc[2][:, tau - 12, :]

            # per-slot pending units: ("p", k) pairs (both widths 512) while
            # k+1 < P0; ("s", k) singles for k = P0..P0+3 (diagonal, masked)
            pend = []
            for s in range(NSLOT):
                u = []
                k = 0
                while k + 1 < P0[s]:
                    u.append(("p", k))
                    k += 2
                while k <= P0[s] + 3:
                    u.append(("s", k))
                    k += 1
                pend.append(u)

            n_o = [0] * 16                       # O-matmuls emitted per q-tile
            tot_o = [P0[t // 4] + (t % 4) + 1 for t in range(16)]
            done_tiles = [0] * NSLOT
            BANK = [range(0, 7), range(7, 12), range(12, 16)]
            bank_left = [len(r) for r in BANK]
            # HW: matmul start=True zeroes the WHOLE PSUM bank, not just the
            # output region.  Only the chronologically-first matmul into each
            # accumulator bank may set start; everything after accumulates.
            bank_virgin = [True, True, True]

            def finalize(tau):
                s = tau // 4
                a = acc_ap(tau)
                rec = mpool.tile([128, 1], f32, tag="rec", name=f"rec_{tau}")
                nc.vector.reciprocal(rec, a[:, HS:HS + 1])
                nc.vector.tensor_scalar_mul(out_sb[:, tau, :], a[:, 0:HS], rec)
                done_tiles[s] += 1
                if done_tiles[s] == 4:
                    # late slots on different queues so tail copies overlap
                    eng = nc.sync if s in (0, 3) else nc.gpsimd
                    eng.dma_start(out=out_d[:, 4 * s:4 * s + 4, :],
                                  in_=out_sb[:, 4 * s:4 * s + 4, :])

            def emit_o(s, t, kk, e_ap):
                tau = 4 * s + t
                b = 0 if tau < 7 else (1 if tau < 12 else 2)
                nc.tensor.matmul(acc_ap(tau), e_ap, vp[:, kk, :],
                                 start=bank_virgin[b],
                                 stop=(n_o[tau] + 1 == tot_o[tau]),
                                 skip_group_check=True)
                bank_virgin[b] = False
                n_o[tau] += 1
                if n_o[tau] == tot_o[tau]:
                    if s == tail_slot:
                        # tail bank (slot-pure) completes tile-by-tile during
                        # the final singles chain; per-tile finalize staggers
                        # the epilogue (tile-WAR serializes the read safely)
                        finalize(tau)
                        return
                    bank_left[b] -= 1
                    if bank_left[b] == 0:
                        for tt in BANK[b]:
                            finalize(tt)

            def emit_unit(s, kind, k):
                base = P0[s]
                if kind == "p":
                    pct = psC.tile([128, 2, 512], f32, tag="pc",
                                   name=f"pc_{s}_{k}")
                    for half, kk in ((0, k), (1, k + 1)):
                        g, c = kk // 4, kk % 4
                        nc.tensor.matmul(
                            pct[:, half, :],
                            kv_sb[0:64, g, c * 128:c * 128 + 128],
                            qT[:, s, :], start=True, stop=True)
                    et = epool.tile([128, 2, 512], bf16, tag="et", bufs=6,
                                    name=f"et_{s}_{k}")
                    nc.scalar.activation(et, pct,
                                         mybir.ActivationFunctionType.Exp)
                    for half, kk in ((0, k), (1, k + 1)):
                        for t in range(4):
                            emit_o(s, t, kk,
                                   et[:, half, t * 128:t * 128 + 128])
                else:
                    off = (k - base) * 128
                    w = 512 - off
                    g, c = k // 4, k % 4
                    pct = psC.tile([128, 2, 512], f32, tag="pc",
                                   name=f"ps_{s}_{k}")
                    nc.tensor.matmul(
                        pct[:, 0, 0:w],
                        kv_sb[0:64, g, c * 128:c * 128 + 128],
                        qT[:, s, off:512], start=True, stop=True)
                    et = epool.tile([128, 512], bf16, tag="ets", bufs=4,
                                    name=f"es_{s}_{k}")
                    nc.scalar.activation(et[:, 0:w], pct[:, 0, 0:w],
                                         mybir.ActivationFunctionType.Exp)
                    nc.vector.tensor_mul(et[:, 0:128], et[:, 0:128], tri)
                    for t in range(k - base, 4):
                        emit_o(s, t, k, et[:, (t * 128 - off):
                                            (t * 128 - off + 128)])

            bpos = {g: p for p, g in enumerate(border)}

            def ready_units(p):
                """(s, kind, k) units emittable at position p: qT present
                (owned block at position <= p), context kv+vp present
                (position < p)."""
                out = []
                for s in range(NSLOT):
                    if bpos[blocks[s]] > p:
                        continue
                    has_pairs = any(kk == "p" for kk, _ in pend[s])
                    for kind, k in pend[s]:
                        if s == tail_slot and kind == "s" and has_pairs:
                            continue   # tail slot: singles close the program
                        klast = k + 1 if kind == "p" else k
                        if bpos[klast // 4] < p:
                            out.append((s, kind, k))
                return out

            # PE clock warmup: the tensor engine ramps to full speed only
            # after ~3us of continuous execution.  PE is idle waiting for the
            # first xt block anyway, so burn that window on dummy matmuls
            # (memset scrap -> scratch PSUM) and start real work at full clock.
            warm = psA.tile([128, 512], f32, tag="pa", name="warm")
            for _ in range(warm_n):
                nc.tensor.matmul(warm[0:64, 0:64], warm_src, warm_src,
                                 start=True, stop=True)

            def _proj_sliced(g, first, out_ap, w_ap, copies):
                """Sliceable projection: returns (run, fin).  run(n) emits n
                c-chunk matmuls (SBUF copies fire as their spans complete);
                fin() drains the rest.  Lets projection work interleave with
                attention units so the PE stream keeps feeding Act."""
                halves = ((0, 256), (256, 512)) if first else ((0, 512),)
                steps = [(lo, hi, ci) for lo, hi in halves
                         for ci in range(CCH)]
                st = {"v": True, "i": 0, "c": 0}

                def run(n):
                    while n > 0 and st["i"] < len(steps):
                        lo, hi, ci = steps[st["i"]]
                        st["i"] += 1
                        n -= 1
                        sl = slice(g * 512 + lo, g * 512 + hi)
                        nc.tensor.matmul(out_ap(lo, hi), w_ap(ci),
                                         xt[:, ci, sl], start=st["v"],
                                         stop=(ci == CCH - 1),
                                         skip_group_check=True)
                        st["v"] = False
                    while st["c"] < len(copies) and \
                            copies[st["c"]][0] <= st["i"]:
                        copies[st["c"]][1]()
                        st["c"] += 1

                def fin():
                    run(len(steps))
                return run, fin

            def proj_kv_sliced(g, first):
                pa = psA.tile([128, 512], f32, tag="pa", name=f"pa_{g}")
                nsteps = 2 * CCH if first else CCH
                copies = [(nsteps, lambda: nc.vector.tensor_copy(
                              kv_sb[:, g, :], pa))]
                return _proj_sliced(g, first,
                                    lambda lo, hi: pa[:, lo:hi], wkv_ap,
                                    copies)

            def proj_qk_sliced(g, first):
                s = blocks.index(g)
                pa = psA.tile([128, 512], f32, tag="pa", name=f"pa_{g}")
                nsteps = 2 * CCH if first else CCH
                if first:
                    # fire the first-half copies while the PE waits on the
                    # second context-half DMA, so V-natural (pa-bank WAR on
                    # these reads) can start right after the last chunk
                    copies = [(CCH, lambda: nc.vector.tensor_copy(
                                  kv_sb[0:64, g, 0:256], pa[0:64, 0:256])),
                              (CCH, lambda: nc.vector.tensor_copy(
                                  qT[:, s, 0:256], pa[64:128, 0:256])),
                              (nsteps, lambda: nc.vector.tensor_copy(
                                  kv_sb[0:64, g, 256:512],
                                  pa[0:64, 256:512])),
                              (nsteps, lambda: nc.vector.tensor_copy(
                                  qT[:, s, 256:512], pa[64:128, 256:512]))]
                else:
                    copies = [(nsteps, lambda: nc.vector.tensor_copy(
                                  kv_sb[0:64, g, :], pa[0:64, :])),
                              (nsteps, lambda: nc.vector.tensor_copy(
                                  qT[:, s, :], pa[64:128, :]))]
                return _proj_sliced(g, first,
                                    lambda lo, hi: pa[:, lo:hi], wqk_ap,
                                    copies)

            def v_nat_sliced(g):
                """V natural for owned block g: V[keys,64] += xt_chunk.T @
                Wv_chunk per 128-key tile, 32 matmuls into the pa bank
                (reused after the K/Q copies drain), one DVE copy to vp."""
                pav = psA.tile([128, 8, 64], f32, tag="pa", name=f"pav_{g}")
                steps = [(i, ci) for i in range(4) for ci in range(CCH)]
                st = {"i": 0}

                def run(n):
                    while n > 0 and st["i"] < len(steps):
                        i, ci = steps[st["i"]]
                        st["i"] += 1
                        n -= 1
                        kc = slice(g * 512 + i * 128, g * 512 + i * 128 + 128)
                        nc.tensor.matmul(pav[:, i, :],
                                         xt[:, ci, kc], wv_mov(ci),
                                         start=(st["i"] == 1),
                                         stop=(ci == CCH - 1),
                                         skip_group_check=True)
                    if st["i"] == len(steps):
                        st["i"] += 1
                        nc.vector.tensor_copy(
                            vp[:, 4 * g:4 * g + 4, 0:HS], pav[:, 0:4, :])

                def fin():
                    run(len(steps))
                return run, fin

            def take_units(p, owned_limit, limit=99):
                todo = [u for u in ready_units(p)
                        if bpos[blocks[u[0]]] <= owned_limit]
                by_slot = [[uu for uu in todo if uu[0] == s]
                           for s in range(NSLOT)]
                rr = []
                while any(by_slot):
                    for s in range(NSLOT):
                        if by_slot[s]:
                            rr.append(by_slot[s].pop(0))
                # singles first: their small exps jump the Act queue so the
                # DVE mask-muls waiting on them don't hold up later copies
                rr = ([u for u in rr if u[1] == "s"]
                      + [u for u in rr if u[1] == "p"])[:limit]
                for u in rr:
                    pend[u[0]].remove((u[1], u[2]))
                return rr

            for p in range(npos + 1):
                # units enabled by ctx/ownership of earlier positions: emit
                # BEFORE this position's projections so the in-order PE queue
                # never stalls on the next DMA block while work is ready.
                # `cap` paces emission so surplus spills into thin positions
                # (scalar, or per-position profile list).
                if p >= npos:
                    budget = 99
                elif isinstance(cap, (list, tuple)):
                    budget = cap[min(p, len(cap) - 1)]
                else:
                    budget = cap
                pre = take_units(p, p - 1, budget)
                budget -= len(pre)
                kv_fin = None
                runs = []
                owned = False
                if p < npos:
                    g = border[p]
                    owned = g in blocks
                    if owned:
                        kv_run, kv_fin = proj_qk_sliced(g, p == 0)
                    else:
                        kv_run, kv_fin = proj_kv_sliced(g, p == 0)
                    runs.append(kv_run)
                # interleave: 3 units up front to absorb DMA-arrival jitter,
                # then alternate units with 4-chunk projection slices so the
                # PE stream keeps producing S-matmuls for the Act engine
                for i, (s, kind, k) in enumerate(pre):
                    emit_unit(s, kind, k)
                    if i >= 2:
                        for r in runs:
                            r(4)
                if kv_fin is not None:
                    kv_fin()
                if p < npos:
                    # units newly enabled by ownership at p (straddle the V
                    # work so the pa-bank WAR on the K/Q copies is covered by
                    # real work)
                    rr = take_units(p, p, max(budget, 1))
                    nfirst = min(2, len(rr))
                    for s, kind, k in rr[:nfirst]:
                        emit_unit(s, kind, k)
                    if owned:
                        v_run, v_fin = v_nat_sliced(g)
                        rest = rr[nfirst:]
                        for j, (s, kind, k) in enumerate(rest):
                            v_run(8)
                            emit_unit(s, kind, k)
                        v_fin()
                    else:
                        # V^T block g -> V natural into vp (PE transpose, pa
                        # bank; first transpose clears the bank, rest
                        # accumulate)
                        vtp = psA.tile([128, 4, HS], bf16, tag="pa",
                                       name=f"vtp_{g}")
                        for i in range(4):
                            nc.tensor.matmul(
                                vtp[:, i, :],
                                kv_sb[64:128, g, i * 128:i * 128 + 128],
                                id_hi[64:128, :], is_transpose=True,
                                start=(i == 0), stop=True,
                                skip_group_check=True)
                        nc.vector.tensor_copy(vp[:, 4 * g:4 * g + 4, 0:HS],
                                              vtp)
                        for s, kind, k in rr[nfirst:]:
                            emit_unit(s, kind, k)

            def emit_singles_burst(rr):
                """Drain-phase tail: fire ALL the S matmuls first (separate
                psC half-banks), then exp/mask/O groups in order — the exp
                latency of single j is covered by the S matmuls of j+1.. and
                the O groups pipeline instead of ping-ponging S->exp->O."""
                pcts = []
                pct = None
                for j, (s, kind, k) in enumerate(rr):
                    off = (k - P0[s]) * 128
                    w = 512 - off
                    g, c = k // 4, k % 4
                    if j % 2 == 0:
                        pct = psC.tile([128, 2, 512], f32, tag="pc",
                                       name=f"psb_{s}_{k}")
                    nc.tensor.matmul(
                        pct[:, j % 2, 0:w],
                        kv_sb[0:64, g, c * 128:c * 128 + 128],
                        qT[:, s, off:512], start=True, stop=True)
                    pcts.append((pct, j % 2, s, off, w, k))
                for pct, half, s, off, w, k in pcts:
                    et = epool.tile([128, 512], bf16, tag="ets", bufs=4,
                                    name=f"esb_{s}_{k}")
                    nc.scalar.activation(et[:, 0:w], pct[:, half, 0:w],
                                         mybir.ActivationFunctionType.Exp)
                    nc.vector.tensor_mul(et[:, 0:128], et[:, 0:128], tri)
                    for t in range(k - P0[s], 4):
                        emit_o(s, t, k, et[:, (t * 128 - off):
                                            (t * 128 - off + 128)])

            # drain: tail-slot singles unlock only after its pairs left pend
            while any(pend):
                rr = take_units(npos, npos)
                assert rr, f"stuck with pending units {pend}"
                if all(kind == "s" for _, kind, _ in rr):
                    emit_singles_burst(rr)
                else:
                    for s, kind, k in rr:
                        emit_unit(s, kind, k)
            assert all(n_o[t] == tot_o[t] for t in range(16)), (n_o, tot_o)

    nc.compile()
    return nc


def _prep_inputs(x, Wq, Wk, Wv):
    bf = ml_dtypes.bfloat16
    wqk = np.concatenate([Wk, Wq * 0.125], axis=1)               # [C, 128]
    wqk_p = wqk.reshape(8, 128, 128).transpose(1, 0, 2)          # [128, 8, 128]
    tri = (np.arange(128)[None, :] >= np.arange(128)[:, None])
    tri = np.broadcast_to(tri.astype(np.float32), (128, 128))[:, None, :]
    wkv = np.concatenate([Wk, Wv], axis=1)                       # [C, 128]
    wkv_p = wkv.reshape(8, 128, 128).transpose(1, 0, 2)          # [128, 8, 128]
    wv_m = Wv.reshape(8, 128, 64).transpose(1, 0, 2)             # [128, 8, 64]
    wv_m = wv_m.reshape(128, 4, 128)
    wpack = np.concatenate(
        [wqk_p, tri, wkv_p, wv_m], axis=1).astype(bf)            # [128, 21, 128]
    in_maps = []
    for core in range(8):
        b = core // 2
        xt = np.ascontiguousarray(x[b].T).astype(bf)
        in_maps.append({"xt": xt, "wpack": wpack})
    return in_maps


def kernel(x, Wq, Wk, Wv):
    from concourse.bass_utils import run_bass_kernel_spmd

    global _programs
    for h in (0, 1):
        if h not in _programs:
            cfg = CONFIG[h]
            _programs[h] = _build_program(cfg["blocks"], cfg["border"],
                                          cfg["sp"], cfg["pool"],
                                          cfg["tail_slot"], cfg["cap"],
                                          cfg["warm"])

    in_maps = _prep_inputs(
        np.asarray(x, np.float32), np.asarray(Wq, np.float32),
        np.asarray(Wk, np.float32), np.asarray(Wv, np.float32),
    )
    out = np.empty((B, T, HS), np.float32)
    res = {}
    res[0] = run_bass_kernel_spmd(_programs[0],
                                  [in_maps[c] for c in (0, 2, 4, 6)],
                                  [0, 2, 4, 6])
    res[1] = run_bass_kernel_spmd(_programs[1],
                                  [in_maps[c] for c in (1, 3, 5, 7)],
                                  [1, 3, 5, 7])
    for core in range(8):
        b, h = core // 2, core % 2
        o = res[h].results[core // 2]["out"]  # [128, 16, 64]
        for j, gblk in enumerate(CONFIG[h]["blocks"]):
            out[b, gblk * 512:(gblk + 1) * 512] = (
                o[:, 4 * j:4 * j + 4, :].transpose(1, 0, 2).reshape(512, HS))
    return out


if __name__ == "__main__":
    rng = np.random.default_rng(0)
    x = rng.standard_normal((B, T, C), dtype=np.float32)
    s = 1 / np.sqrt(C)
    Wq = rng.standard_normal((C, HS), dtype=np.float32) * s
    Wk = rng.standard_normal((C, HS), dtype=np.float32) * s
    Wv = rng.standard_normal((C, HS), dtype=np.float32) * s
    o = kernel(x=x, Wq=Wq, Wk=Wk, Wv=Wv)
    print(o.shape, o.dtype, np.abs(o).mean())
